# revision 1
# baseline (speedup 1.0000x reference)
"""KascadeReuseAttention Trainium2 kernel.

Sharding: 16 heads / 8 cores -> 2 heads per core (head/tensor parallel).
Wq/Wk/Wv column-sharded by head, Wo row-sharded; host sums the 8 partial
outputs (the row-parallel all-reduce).

Single SPMD program for all cores: per-core anchor selection enters only as
DATA (per-tile multiplicity weight columns), never as program structure.
Per (head, query-tile t) we compute block attention against ALL past key
tiles v<=t and weight each tile's probabilities by m[h,t,v] = multiplicity
of v in {anchors} + {t} (0 if unselected) before the denominator and PV
matmuls. This reproduces the reference exactly (duplicate anchors included)
while keeping K/V resident in SBUF (no DRAM gather round trip).
"""

import math
import sys

import numpy as np

for _p in ("/opt/trn_rl_repo",):
    if _p not in sys.path:
        sys.path.insert(0, _p)

import ml_dtypes  # noqa: E402
import concourse.bass as bass  # noqa: E402
import concourse.mybir as mybir  # noqa: E402
import concourse.tile as tile  # noqa: E402
from concourse.bass_utils import run_bass_kernel_spmd  # noqa: E402
from concourse.vector_clock import ScopedClock  # noqa: E402

BF16 = mybir.dt.bfloat16
F32 = mybir.dt.float32
NPBF16 = ml_dtypes.bfloat16

B, S, E, H, D, K = 1, 4096, 2048, 16, 128, 8
TILE = 128
T = S // TILE          # 32 query/key tiles
NCORES = 8
HPC = H // NCORES      # heads per core = 2
CHUNK = 512            # s-chunk for projections
NCHUNK = S // CHUNK
SM_SCALE = 1.0 / math.sqrt(D)
NTRI = T * (T + 1) // 2  # 528 (t,v<=t) pairs per head

_PATCHED = False


def _patch_tile_drain():
    """This container's walrus caps per-instruction sync waits; the Tile
    kernel-tail drain carries one wait per live semaphore. Split them onto
    preceding SP nops."""
    global _PATCHED
    if _PATCHED:
        return
    _PATCHED = True

    def _drain_and_barrier(self, tick_clock, wait_clock):
        nc = self.nc
        nops = []
        nsems = len(self.sems.allocated()) if self.sems is not None else 0
        for _ in range(nsems):
            nops.append(nc.sync.nop())
        drain_inst = nc.sync.drain()
        wait_clock.add_sem_waits(
            drain_inst.ins, ScopedClock({None: tick_clock.global_clock})
        )
        si = drain_inst.ins.sync_info
        waits = list(si.on_wait or [])
        if len(waits) > 1:
            si.on_wait = waits[:1]
            for i, w in enumerate(waits[1:]):
                ni = nops[i].ins if hasattr(nops[i], "ins") else nops[i]
                nsi = ni.sync_info
                if nsi is None:
                    ni.sync_info = mybir.SyncInfo(on_wait=[w], on_update=[])
                else:
                    nsi.on_wait = [w]
        nc.all_engine_barrier()
        assert self.sems is not None
        popped = nc._tile_sem_poison_stack.pop()
        assert popped is self._sem_poison
        nc.clear_and_free_semaphores(list(self.sems.allocated().values()))
        nc.all_engine_barrier()
        _split_multi_waits(nc)

    tile.TileContext._drain_and_barrier = _drain_and_barrier


def _split_multi_waits(nc):
    """Walrus here encodes at most one sync-wait per instruction; move the
    extras onto preceding same-engine no-ops."""
    ctr = [0]
    for f in nc.m.functions:
        for bb in f.blocks:
            insts = list(bb.instructions)
            if not any(
                i.sync_info and i.sync_info.on_wait
                and len(i.sync_info.on_wait) > 1
                for i in insts
            ):
                continue
            newl = []
            for inst in insts:
                si = inst.sync_info
                if si and si.on_wait and len(si.on_wait) > 1:
                    waits = list(si.on_wait)
                    for w in waits[:-1]:
                        ctr[0] += 1
                        nop = mybir.InstNoOp(
                            name=f"WSPL-{ctr[0]}", ins=[], outs=[])
                        nop.engine = inst.engine
                        nop.sync_info = mybir.SyncInfo(
                            on_wait=[w], on_update=[])
                        newl.append(nop)
                    si.on_wait = waits[-1:]
                newl.append(inst)
            bb.instructions = newl


def _tri_col(t, v):
    return t * (t + 1) // 2 + v


def build_bass():
    """Uniform per-core program. Inputs (per core, bf16 unless noted):
    xT [E, S], wqk [E, 4*128] (q_h0,q_h1,k_h0,k_h1), wv [E, 256],
    wo [256, E], cosT/sinT [128, S], rotT [128,128] (R^T for rotate_half),
    triT [128,128] (tri[l,q] = l<=q), mw [128, HPC*NTRI] f32 weight columns.
    Output: outT [E, S] f32 (partial x@.. contribution of this core's heads).
    """
    nc = bass.Bass()
    xT = nc.dram_tensor("xT", [E, S], BF16, kind="ExternalInput")
    wqk = nc.dram_tensor("wqk", [E, 4 * TILE], BF16, kind="ExternalInput")
    wv = nc.dram_tensor("wv", [E, 2 * TILE], BF16, kind="ExternalInput")
    wo = nc.dram_tensor("wo", [2 * TILE, E], BF16, kind="ExternalInput")
    cosT = nc.dram_tensor("cosT", [TILE, S], BF16, kind="ExternalInput")
    sinT = nc.dram_tensor("sinT", [TILE, S], BF16, kind="ExternalInput")
    rotT = nc.dram_tensor("rotT", [TILE, TILE], BF16, kind="ExternalInput")
    triT = nc.dram_tensor("triT", [TILE, TILE], BF16, kind="ExternalInput")
    mw = nc.dram_tensor("mw", [TILE, HPC * NTRI], F32, kind="ExternalInput")
    outT = nc.dram_tensor("outT", [E, S], F32, kind="ExternalOutput")

    EK = E // TILE  # 16 contraction tiles

    with tile.TileContext(nc) as tc:
        with tc.tile_pool(name="const", bufs=1) as cpool:
            sb_wqk = cpool.tile([TILE, EK, 4 * TILE], BF16)
            sb_wv = cpool.tile([TILE, EK, 2 * TILE], BF16)
            sb_wo = cpool.tile([TILE, 2, E], BF16)
            sb_cos = cpool.tile([TILE, S], BF16)
            sb_sin = cpool.tile([TILE, S], BF16)
            sb_rot = cpool.tile([TILE, TILE], BF16)
            sb_tri = cpool.tile([TILE, TILE], BF16)
            sb_mw = cpool.tile([TILE, HPC * NTRI], F32)
            ones_col = cpool.tile([TILE, 1], BF16)
            ones_row = cpool.tile([1, TILE], F32)
            # persistent per-head tensors (bf16): qT/kT [d, S], v [s-tiles, d]
            sb_q = cpool.tile([TILE, HPC, S], BF16, tag="q")
            sb_k = cpool.tile([TILE, HPC, S], BF16, tag="k")
            sb_v = cpool.tile([TILE, HPC, S], BF16, tag="v")
            sb_attn = cpool.tile([TILE, HPC, S], BF16, tag="attn")

            nc.sync.dma_start(out=sb_wqk[:],
                              in_=wqk.rearrange("(a p) b -> p a b", p=TILE))
            nc.sync.dma_start(out=sb_wv[:],
                              in_=wv.rearrange("(a p) b -> p a b", p=TILE))
            nc.sync.dma_start(out=sb_wo[:],
                              in_=wo.rearrange("(a p) b -> p a b", p=TILE))
            nc.sync.dma_start(out=sb_cos[:], in_=cosT[:])
            nc.sync.dma_start(out=sb_sin[:], in_=sinT[:])
            nc.sync.dma_start(out=sb_rot[:], in_=rotT[:])
            nc.sync.dma_start(out=sb_tri[:], in_=triT[:])
            nc.sync.dma_start(out=sb_mw[:], in_=mw[:])
            nc.vector.memset(ones_col[:], 1.0)
            nc.vector.memset(ones_row[:], 1.0)

            # ---------------- Phase 1: projections + RoPE ----------------
            with (
                tc.tile_pool(name="xin", bufs=2) as xpool,
                tc.tile_pool(name="ptmp", bufs=3) as tpool,
                tc.tile_pool(name="qkps", bufs=2, space="PSUM") as qkps,
                tc.tile_pool(name="vps", bufs=2, space="PSUM") as vps,
                tc.tile_pool(name="rops", bufs=2, space="PSUM") as rops,
            ):
                for ci in range(NCHUNK):
                    s0 = ci * CHUNK
                    xt = xpool.tile([TILE, EK, CHUNK], BF16, tag="xt")
                    nc.sync.dma_start(
                        out=xt[:],
                        in_=xT[:, s0:s0 + CHUNK].rearrange(
                            "(a p) b -> p a b", p=TILE),
                    )
                    # qT/kT M-tiles: 0=q_h0 1=q_h1 2=k_h0 3=k_h1
                    for m in range(4):
                        ps = qkps.tile([TILE, CHUNK], F32, tag="qk")
                        for e in range(EK):
                            nc.tensor.matmul(
                                ps[:], sb_wqk[:, e, m * TILE:(m + 1) * TILE],
                                xt[:, e, :], start=(e == 0), stop=(e == EK - 1))
                        raw = tpool.tile([TILE, CHUNK], BF16, tag="raw")
                        nc.scalar.copy(out=raw[:], in_=ps[:])
                        rot = rops.tile([TILE, CHUNK], F32, tag="rot")
                        nc.tensor.matmul(rot[:], sb_rot[:], raw[:],
                                         start=True, stop=True)
                        t1 = tpool.tile([TILE, CHUNK], BF16, tag="t1")
                        nc.vector.tensor_mul(t1[:], raw[:],
                                             sb_cos[:, s0:s0 + CHUNK])
                        t2 = tpool.tile([TILE, CHUNK], BF16, tag="t2")
                        nc.vector.tensor_mul(t2[:], rot[:],
                                             sb_sin[:, s0:s0 + CHUNK])
                        dst = sb_q if m < 2 else sb_k
                        h = m % 2
                        nc.vector.tensor_add(dst[:, h, s0:s0 + CHUNK],
                                             t1[:], t2[:])
                    # v: M-tiles over s (4 per chunk), N = 2 heads * 128
                    for sm in range(CHUNK // TILE):
                        vp = vps.tile([TILE, 2 * TILE], F32, tag="v")
                        st = s0 + sm * TILE
                        for e in range(EK):
                            nc.tensor.matmul(
                                vp[:], xt[:, e, st - s0 + 0:st - s0 + TILE],
                                sb_wv[:, e, :], start=(e == 0),
                                stop=(e == EK - 1))
                        for h in range(HPC):
                            nc.scalar.copy(
                                out=sb_v[:, h, st:st + TILE],
                                in_=vp[:, h * TILE:(h + 1) * TILE])

            # ---------------- Phase 2: block-sparse attention ------------
            GRP = 4  # logits tiles per psum bank
            with (
                tc.tile_pool(name="wt", bufs=3) as wtp,
                tc.tile_pool(name="nrm", bufs=3) as nrm,
                tc.tile_pool(name="lg", bufs=2, space="PSUM") as lgps,
                tc.tile_pool(name="ot", bufs=2, space="PSUM") as otps,
                tc.tile_pool(name="dn", bufs=2, space="PSUM") as dnps,
                tc.tile_pool(name="bc", bufs=2, space="PSUM") as bcps,
            ):
                for h in range(HPC):
                    for t in range(T):
                        nv = t + 1
                        q_sl = sb_q[:, h, t * TILE:(t + 1) * TILE]
                        out_ps = otps.tile([TILE, TILE], F32, tag="ot")
                        den_ps = dnps.tile([1, TILE], F32, tag="dn")
                        ngrp = (nv + GRP - 1) // GRP
                        first = True
                        for g in range(ngrp):
                            v0 = g * GRP
                            gn = min(GRP, nv - v0)
                            lg = lgps.tile([TILE, GRP * TILE], F32, tag="lg")
                            for j in range(gn):
                                v = v0 + j
                                nc.tensor.matmul(
                                    lg[:, j * TILE:(j + 1) * TILE],
                                    sb_k[:, h, v * TILE:(v + 1) * TILE],
                                    q_sl, start=True, stop=True)
                            wt = wtp.tile([TILE, GRP * TILE], BF16, tag="wt")
                            nc.scalar.activation(
                                out=wt[:, :gn * TILE], in_=lg[:, :gn * TILE],
                                func=mybir.ActivationFunctionType.Exp,
                                scale=SM_SCALE)
                            for j in range(gn):
                                v = v0 + j
                                mcol = sb_mw[:, h * NTRI + _tri_col(t, v):
                                             h * NTRI + _tri_col(t, v) + 1]
                                wsl = wt[:, j * TILE:(j + 1) * TILE]
                                if v == t:
                                    nc.vector.scalar_tensor_tensor(
                                        out=wsl, in0=wsl, scalar=mcol,
                                        in1=sb_tri[:],
                                        op0=mybir.AluOpType.mult,
                                        op1=mybir.AluOpType.mult)
                                else:
                                    nc.vector.tensor_scalar_mul(wsl, wsl, mcol)
                            for j in range(gn):
                                v = v0 + j
                                last = (g == ngrp - 1) and (j == gn - 1)
                                wsl = wt[:, j * TILE:(j + 1) * TILE]
                                nc.tensor.matmul(
                                    den_ps[:], ones_col[:], wsl,
                                    start=first, stop=last,
                                    skip_group_check=True)
                                nc.tensor.matmul(
                                    out_ps[:],
                                    sb_v[:, h, v * TILE:(v + 1) * TILE],
                                    wsl, start=first, stop=last,
                                    skip_group_check=True)
                                first = False
                        # normalize: recip(denom) -> broadcast -> multiply
                        rc = nrm.tile([1, TILE], F32, tag="rc")
                        nc.vector.reciprocal(out=rc[:], in_=den_ps[:])
                        bc = bcps.tile([TILE, TILE], F32, tag="bc")
                        nc.tensor.matmul(bc[:], ones_row[:], rc[:],
                                         start=True, stop=True)
                        un = nrm.tile([TILE, TILE], BF16, tag="un")
                        nc.scalar.copy(out=un[:], in_=out_ps[:])
                        nc.vector.tensor_mul(
                            sb_attn[:, h, t * TILE:(t + 1) * TILE],
                            un[:], bc[:])

            # ---------------- Phase 3: output projection -----------------
            with (
                tc.tile_pool(name="ost", bufs=3) as ost,
                tc.tile_pool(name="wops", bufs=4, space="PSUM") as wops,
            ):
                for ci in range(NCHUNK):
                    s0 = ci * CHUNK
                    for m in range(EK):  # output e tiles
                        op = wops.tile([TILE, CHUNK], F32, tag="op")
                        for h in range(HPC):
                            nc.tensor.matmul(
                                op[:], sb_wo[:, h, m * TILE:(m + 1) * TILE],
                                sb_attn[:, h, s0:s0 + CHUNK],
                                start=(h == 0), stop=(h == HPC - 1))
                        ob = ost.tile([TILE, CHUNK], F32, tag="ob")
                        if m % 2 == 0:
                            nc.scalar.copy(out=ob[:], in_=op[:])
                        else:
                            nc.vector.tensor_copy(ob[:], op[:])
                        nc.sync.dma_start(
                            out=outT[m * TILE:(m + 1) * TILE, s0:s0 + CHUNK],
                            in_=ob[:])
    return nc


def _host_prep(x, wq, wk, wv, wo, rope_angles, anchor_indices):
    xT = np.ascontiguousarray(x[0].T).astype(NPBF16)
    cos = np.cos(rope_angles.astype(np.float64))
    sin = np.sin(rope_angles.astype(np.float64))
    cosT = np.ascontiguousarray(
        np.concatenate([cos, cos], axis=1).T).astype(NPBF16)
    sinT = np.ascontiguousarray(
        np.concatenate([sin, sin], axis=1).T).astype(NPBF16)
    half = D // 2
    R = np.zeros((D, D), np.float32)
    for d in range(half):
        R[d, d + half] = -1.0
        R[d + half, d] = 1.0
    rotT = np.ascontiguousarray(R.T).astype(NPBF16)
    tri = (np.arange(TILE)[:, None] <= np.arange(TILE)[None, :])
    triT = tri.astype(NPBF16)

    in_maps = []
    for c in range(NCORES):
        heads = [c * HPC + i for i in range(HPC)]
        wqk_c = np.concatenate(
            [wq[:, h * D:(h + 1) * D] for h in heads]
            + [wk[:, h * D:(h + 1) * D] for h in heads], axis=1)
        wv_c = np.concatenate([wv[:, h * D:(h + 1) * D] for h in heads],
                              axis=1)
        wo_c = np.concatenate([wo[h * D:(h + 1) * D, :] for h in heads],
                              axis=0)
        mwc = np.zeros((TILE, HPC * NTRI), np.float32)
        for i, h in enumerate(heads):
            for t in range(T):
                sel = list(anchor_indices[0, h, t]) + [t]
                for v in range(t + 1):
                    m = sel.count(v)
                    if m:
                        mwc[:, i * NTRI + _tri_col(t, v)] = float(m)
        in_maps.append({
            "xT": xT, "wqk": np.ascontiguousarray(wqk_c).astype(NPBF16),
            "wv": np.ascontiguousarray(wv_c).astype(NPBF16),
            "wo": np.ascontiguousarray(wo_c).astype(NPBF16),
            "cosT": cosT, "sinT": sinT, "rotT": rotT, "triT": triT,
            "mw": mwc,
        })
    return in_maps


def kernel(x, wq, wk, wv, wo, rope_angles, anchor_indices, **run_kwargs):
    _patch_tile_drain()
    nc = build_bass()
    in_maps = _host_prep(x, wq, wk, wv, wo, rope_angles, anchor_indices)
    res = run_bass_kernel_spmd(nc, in_maps, core_ids=list(range(NCORES)),
                               **run_kwargs)
    acc = np.zeros((E, S), np.float64)
    for c in range(NCORES):
        acc += res.results[c]["outT"].astype(np.float64)
    out = np.ascontiguousarray(acc.T.reshape(B, S, E)).astype(np.float32)
    kernel.last_results = res
    return out



# revision 20
# speedup vs baseline: 1.6789x; 1.6789x over previous
"""KascadeReuseAttention Trainium2 kernel (v2).

Sharding: 16 heads / 8 cores -> 2 heads per core (head/tensor parallel).
Wq/Wk/Wv column-sharded by head, Wo row-sharded; host sums the 8 partial
outputs (the row-parallel all-reduce).

Single SPMD program for all cores: per-core anchor selection enters only as
DATA. Per (head, query-tile t) we compute block attention against ALL past
key tiles v<=t; tile multiplicities m[h,t,v] (count of v among anchors+local,
0 if unselected) are folded into the LOGITS as additive biases
B = sqrt(D)*ln(m) (-4e9 when m=0), accumulated into the logits PSUM by a
tiny K=1 matmul reading a host-precomputed bias row. exp() then yields
m*exp(s*qk) (or 0) with no per-pair vector work. The causal tri-mask for the
diagonal tile is likewise one additive matmul (identity x trineg).

The denominator rides in the PV matmul: V tiles carry an appended
ones-column, so PV produces [q, D+1] with the last column = sum of weights.
Normalization is then a per-partition reciprocal + scalar multiply, and the
[q, d] -> [d, q] transpose for the output projection is a DMA transpose.
"""

import math
import sys

import numpy as np

for _p in ("/opt/trn_rl_repo",):
    if _p not in sys.path:
        sys.path.insert(0, _p)

import ml_dtypes  # noqa: E402
import concourse.bass as bass  # noqa: E402
import concourse.mybir as mybir  # noqa: E402
import concourse.tile as tile  # noqa: E402
from concourse.bass_utils import run_bass_kernel_spmd  # noqa: E402
from concourse.vector_clock import ScopedClock  # noqa: E402

BF16 = mybir.dt.bfloat16
F32 = mybir.dt.float32
NPBF16 = ml_dtypes.bfloat16

B, S, E, H, D, K = 1, 4096, 2048, 16, 128, 8
TILE = 128
T = S // TILE          # 32 query/key tiles
NCORES = 8
HPC = H // NCORES      # heads per core = 2
CHUNK = 512            # s-chunk for projections
NCHUNK = S // CHUNK
EK = E // TILE         # 16 contraction tiles
SM_SCALE = 1.0 / math.sqrt(D)
GRP = 4                # logits tiles per psum bank
NEGB = -4e9            # additive bias for m=0 pairs (exp -> 0)
NEGT = -1e9            # additive causal mask value

_PATCHED = False


def _patch_tile_drain():
    """This container's walrus caps per-instruction sync waits; the Tile
    kernel-tail drain carries one wait per live semaphore. Split them onto
    preceding SP nops."""
    global _PATCHED
    if _PATCHED:
        return
    _PATCHED = True

    def _drain_and_barrier(self, tick_clock, wait_clock):
        nc = self.nc
        nops = []
        nsems = len(self.sems.allocated()) if self.sems is not None else 0
        for _ in range(nsems):
            nops.append(nc.sync.nop())
        drain_inst = nc.sync.drain()
        wait_clock.add_sem_waits(
            drain_inst.ins, ScopedClock({None: tick_clock.global_clock})
        )
        si = drain_inst.ins.sync_info
        waits = list(si.on_wait or [])
        if len(waits) > 1:
            si.on_wait = waits[:1]
            for i, w in enumerate(waits[1:]):
                ni = nops[i].ins if hasattr(nops[i], "ins") else nops[i]
                nsi = ni.sync_info
                if nsi is None:
                    ni.sync_info = mybir.SyncInfo(on_wait=[w], on_update=[])
                else:
                    nsi.on_wait = [w]
        nc.all_engine_barrier()
        assert self.sems is not None
        popped = nc._tile_sem_poison_stack.pop()
        assert popped is self._sem_poison
        nc.clear_and_free_semaphores(list(self.sems.allocated().values()))
        nc.all_engine_barrier()
        _split_multi_waits(nc)

    tile.TileContext._drain_and_barrier = _drain_and_barrier


def _split_multi_waits(nc):
    """Walrus here encodes at most one sync-wait per instruction; move the
    extras onto preceding same-engine no-ops."""
    ctr = [0]
    for f in nc.m.functions:
        for bb in f.blocks:
            insts = list(bb.instructions)
            if not any(
                i.sync_info and i.sync_info.on_wait
                and len(i.sync_info.on_wait) > 1
                for i in insts
            ):
                continue
            newl = []
            for inst in insts:
                si = inst.sync_info
                if si and si.on_wait and len(si.on_wait) > 1:
                    waits = list(si.on_wait)
                    for w in waits[:-1]:
                        ctr[0] += 1
                        nop = mybir.InstNoOp(
                            name=f"WSPL-{ctr[0]}", ins=[], outs=[])
                        nop.engine = inst.engine
                        nop.sync_info = mybir.SyncInfo(
                            on_wait=[w], on_update=[])
                        newl.append(nop)
                    si.on_wait = waits[-1:]
                newl.append(inst)
            bb.instructions = newl


def build_bass():
    """Uniform per-core program. Inputs (per core, bf16 unless noted):
    xT [E, S], wqk [E, 4*128] (q_h0,q_h1,k_h0,k_h1), wv [E, 256],
    wo [256, E], cosT/sinT [128, S], rotT [128,128] (R^T for rotate_half),
    ident [128,128], trineg [128,128] (additive causal mask, -1e9 below
    diag), mwx [128, T*128] (bias rows: partition h*32+t holds, at col
    j*128+i, the value sqrt(D)*ln(m[h,t,j]) or -4e9).
    Output: outT [E, S] bf16 (partial contribution of this core's heads).
    """
    nc = bass.Bass()
    xT = nc.dram_tensor("xT", [E, S], BF16, kind="ExternalInput")
    wqk = nc.dram_tensor("wqk", [E, 4 * TILE], BF16, kind="ExternalInput")
    wv = nc.dram_tensor("wv", [E, 2 * TILE], BF16, kind="ExternalInput")
    wo = nc.dram_tensor("wo", [2 * TILE, E], BF16, kind="ExternalInput")
    cosT = nc.dram_tensor("cosT", [TILE, S], BF16, kind="ExternalInput")
    sinT = nc.dram_tensor("sinT", [TILE, S], BF16, kind="ExternalInput")
    rotT = nc.dram_tensor("rotT", [TILE, TILE], BF16, kind="ExternalInput")
    identD = nc.dram_tensor("identD", [TILE, TILE], BF16,
                            kind="ExternalInput")
    trinegD = nc.dram_tensor("trinegD", [TILE, TILE], BF16,
                             kind="ExternalInput")
    mwxD = nc.dram_tensor("mwxD", [2 * T, T * TILE], BF16,
                          kind="ExternalInput")
    ohD = nc.dram_tensor("ohD", [2 * T, T * TILE], BF16,
                         kind="ExternalInput")
    outT = nc.dram_tensor("outT", [E, S], BF16, kind="ExternalOutput")

    with tile.TileContext(nc) as tc:
        with tc.tile_pool(name="const", bufs=1) as cpool:
            sb_wqk = cpool.tile([TILE, EK, 4 * TILE], BF16)
            sb_wv = cpool.tile([TILE, EK, 2 * TILE], BF16)
            sb_wo = cpool.tile([TILE, 2, E], BF16)
            sb_cos = cpool.tile([TILE, S], BF16)
            sb_sin = cpool.tile([TILE, S], BF16)
            sb_rot = cpool.tile([TILE, TILE], BF16)
            sb_id = cpool.tile([TILE, TILE], BF16)
            sb_tn = cpool.tile([TILE, TILE], BF16)
            sb_mwx = cpool.tile([2 * T, T * TILE], BF16)
            sb_oh = cpool.tile([2 * T, T * TILE], BF16)
            # persistent per-head tensors: qT/kT [d, S]; v [kv, 132] per
            # tile with col 128 = 1.0 (denominator column); attnT [d, S]
            sb_q = cpool.tile([TILE, HPC, S], BF16, tag="q")
            sb_k = cpool.tile([TILE, HPC, S], BF16, tag="k")
            sb_vg = cpool.tile([TILE, HPC, T, 132], BF16, tag="vg")
            sb_attn = cpool.tile([TILE, HPC, S], BF16, tag="attn")

            nc.sync.dma_start(out=sb_wqk[:],
                              in_=wqk.rearrange("(a p) b -> p a b", p=TILE))
            nc.sync.dma_start(out=sb_wv[:],
                              in_=wv.rearrange("(a p) b -> p a b", p=TILE))
            nc.sync.dma_start(out=sb_wo[:],
                              in_=wo.rearrange("(a p) b -> p a b", p=TILE))
            nc.sync.dma_start(out=sb_cos[:], in_=cosT[:])
            nc.sync.dma_start(out=sb_sin[:], in_=sinT[:])
            nc.sync.dma_start(out=sb_rot[:], in_=rotT[:])
            nc.sync.dma_start(out=sb_id[:], in_=identD[:])
            nc.sync.dma_start(out=sb_tn[:], in_=trinegD[:])
            nc.sync.dma_start(out=sb_mwx[:], in_=mwxD[:])
            nc.sync.dma_start(out=sb_oh[:], in_=ohD[:])
            nc.vector.memset(sb_vg[:, :, :, 128:129], 1.0)

            with (
                tc.tile_pool(name="xin", bufs=2) as xpool,
                tc.tile_pool(name="rawp", bufs=3) as rawp,
                tc.tile_pool(name="t1p", bufs=3) as t1p,
                tc.tile_pool(name="t2p", bufs=3) as t2p,
                tc.tile_pool(name="wtp", bufs=3) as wtp,
                tc.tile_pool(name="nrm", bufs=3) as nrm,
                tc.tile_pool(name="obp", bufs=2) as obp,
                tc.tile_pool(name="pp", bufs=3, space="PSUM") as pp,
                tc.tile_pool(name="lg", bufs=2, space="PSUM") as lgp_pool,
                tc.tile_pool(name="oo", bufs=2, space="PSUM") as oo,
                tc.tile_pool(name="trp", bufs=1, space="PSUM") as trp,
            ):
                for ci in range(NCHUNK):
                    s0 = ci * CHUNK
                    xt = xpool.tile([TILE, EK, CHUNK], BF16, tag="xt")
                    nc.sync.dma_start(
                        out=xt[:],
                        in_=xT[:, s0:s0 + CHUNK].rearrange(
                            "(a p) b -> p a b", p=TILE),
                    )
                    # ---- projections + RoPE for this chunk ----
                    # qT/kT M-tiles: 0=q_h0 1=q_h1 2=k_h0 3=k_h1
                    for m in range(4):
                        ps = pp.tile([TILE, CHUNK], F32, tag="ps")
                        for e in range(EK):
                            nc.tensor.matmul(
                                ps[:], sb_wqk[:, e, m * TILE:(m + 1) * TILE],
                                xt[:, e, :], start=(e == 0), stop=(e == EK - 1))
                        raw = rawp.tile([TILE, CHUNK], BF16, tag="raw")
                        nc.scalar.copy(out=raw[:], in_=ps[:])
                        rot = pp.tile([TILE, CHUNK], F32, tag="ps")
                        nc.tensor.matmul(rot[:], sb_rot[:], raw[:],
                                         start=True, stop=True)
                        t1 = t1p.tile([TILE, CHUNK], BF16, tag="t1")
                        nc.gpsimd.tensor_mul(t1[:], raw[:],
                                             sb_cos[:, s0:s0 + CHUNK])
                        t2 = t2p.tile([TILE, CHUNK], BF16, tag="t2")
                        nc.vector.tensor_mul(t2[:], rot[:],
                                             sb_sin[:, s0:s0 + CHUNK])
                        dst = sb_q if m < 2 else sb_k
                        h = m % 2
                        nc.vector.tensor_add(dst[:, h, s0:s0 + CHUNK],
                                             t1[:], t2[:])
                    # v: M-tiles over s (4 per chunk), N = 2 heads * 128
                    for sm in range(CHUNK // TILE):
                        vp = pp.tile([TILE, CHUNK], F32, tag="ps")
                        st = sm * TILE
                        for e in range(EK):
                            nc.tensor.matmul(
                                vp[:, :2 * TILE], xt[:, e, st:st + TILE],
                                sb_wv[:, e, :], start=(e == 0),
                                stop=(e == EK - 1))
                        vt = ci * 4 + sm
                        for h in range(HPC):
                            nc.vector.tensor_copy(
                                sb_vg[:, h, vt, 0:TILE],
                                vp[:, h * TILE:(h + 1) * TILE])

                    # ---- block-sparse attention for this chunk's tiles ----
                    for tt in range(4):
                        t = ci * 4 + tt
                        for h in range(HPC):
                            _attend(nc, t, h, sb_q, sb_k, sb_vg, sb_attn,
                                    sb_mwx, sb_id, sb_tn, sb_oh,
                                    lgp_pool, oo, trp, wtp, nrm)

                    # ---- output projection for this chunk ----
                    ob = obp.tile([TILE, EK, CHUNK], BF16, tag="ob")
                    for m in range(EK):
                        op = pp.tile([TILE, CHUNK], F32, tag="ps")
                        for h in range(HPC):
                            nc.tensor.matmul(
                                op[:], sb_wo[:, h, m * TILE:(m + 1) * TILE],
                                sb_attn[:, h, s0:s0 + CHUNK],
                                start=(h == 0), stop=(h == HPC - 1))
                        if m % 2 == 0:
                            nc.scalar.copy(out=ob[:, m, :], in_=op[:])
                        else:
                            nc.vector.tensor_copy(ob[:, m, :], op[:])
                    nc.sync.dma_start(
                        out=outT[:, s0:s0 + CHUNK].rearrange(
                            "(a p) b -> p a b", p=TILE),
                        in_=ob[:])
    return nc


def _attend(nc, t, h, sb_q, sb_k, sb_vg, sb_attn, sb_mwx, sb_id, sb_tn,
            sb_oh, lgp_pool, oo, trp, wtp, nrm):
    """Attention for one (head, query tile): logits groups of GRP tiles with
    additive bias/mask matmuls, exp, PV with denominator column, normalize,
    DMA-transpose into sb_attn[d, s]."""
    q_sl = sb_q[:, h, t * TILE:(t + 1) * TILE]
    h0 = h * T
    nv = t + 1
    ngrp = (nv + GRP - 1) // GRP
    out_ps = oo.tile([TILE, 132], F32, tag="oo")

    # software pipeline: stage QK/bias for group g, PV trails by PIPE groups
    PIPE = 1
    lgs = [None] * ngrp
    wts = [None] * ngrp

    def emit_qk(g):
        v0 = g * GRP
        gn = min(GRP, nv - v0)
        lg = lgp_pool.tile([TILE, GRP * TILE], F32, tag="lg")
        lgs[g] = lg
        has_diag = v0 <= t < v0 + gn
        # additive multiplicity bias FIRST: start=True clears the whole
        # bank's has_written bits, so it must precede the accumulating QK
        # segment matmuls. One K=32 matmul over the group; the one-hot
        # lhsT selects row t of head h's 32 bias rows.
        nc.tensor.matmul(
            lg[:, :gn * TILE],
            sb_oh[h0:h0 + T, t * TILE:(t + 1) * TILE],
            sb_mwx[h0:h0 + T, v0 * TILE:(v0 + gn) * TILE],
            start=True, stop=False, skip_group_check=True)
        for j in range(gn):
            v = v0 + j
            nc.tensor.matmul(
                lg[:, j * TILE:(j + 1) * TILE],
                sb_k[:, h, v * TILE:(v + 1) * TILE],
                q_sl, start=False,
                stop=(j == gn - 1 and not has_diag),
                skip_group_check=True)
        if has_diag:
            j = t - v0
            nc.tensor.matmul(
                lg[:, j * TILE:(j + 1) * TILE], sb_id[:], sb_tn[:],
                start=False, stop=True, skip_group_check=True)
        wt = wtp.tile([TILE, GRP * TILE], BF16, tag="wt")
        wts[g] = wt
        nc.scalar.activation(
            out=wt[:, :gn * TILE], in_=lg[:, :gn * TILE],
            func=mybir.ActivationFunctionType.Exp, scale=SM_SCALE)

    def emit_pv(g):
        v0 = g * GRP
        gn = min(GRP, nv - v0)
        wt = wts[g]
        for j in range(gn):
            v = v0 + j
            nc.tensor.matmul(
                out_ps[:, :129], wt[:, j * TILE:(j + 1) * TILE],
                sb_vg[:, h, v, 0:129],
                start=(v == 0), stop=(v == t), skip_group_check=True)

    for g in range(min(PIPE, ngrp)):
        emit_qk(g)
    for g in range(ngrp):
        if g + PIPE < ngrp:
            emit_qk(g + PIPE)
        emit_pv(g)

    rc = nrm.tile([TILE, 1], F32, tag="rc")
    nc.vector.reciprocal(out=rc[:], in_=out_ps[:, 128:129])
    an = nrm.tile([TILE, TILE], BF16, tag="an")
    nc.vector.tensor_scalar_mul(an[:], out_ps[:, 0:TILE], rc[:])
    tr = trp.tile([TILE, TILE], BF16, tag="tr")
    nc.tensor.transpose(tr[:], an[:], sb_id[:])
    nc.scalar.copy(out=sb_attn[:, h, t * TILE:(t + 1) * TILE], in_=tr[:])


def _host_prep(x, wq, wk, wv, wo, rope_angles, anchor_indices):
    xT = np.ascontiguousarray(x[0].T).astype(NPBF16)
    cos = np.cos(rope_angles.astype(np.float64))
    sin = np.sin(rope_angles.astype(np.float64))
    cosT = np.ascontiguousarray(
        np.concatenate([cos, cos], axis=1).T).astype(NPBF16)
    sinT = np.ascontiguousarray(
        np.concatenate([sin, sin], axis=1).T).astype(NPBF16)
    half = D // 2
    R = np.zeros((D, D), np.float32)
    for d in range(half):
        R[d, d + half] = -1.0
        R[d + half, d] = 1.0
    rotT = np.ascontiguousarray(R.T).astype(NPBF16)
    ident = np.eye(TILE, dtype=np.float32).astype(NPBF16)
    trineg = np.where(np.arange(TILE)[:, None] > np.arange(TILE)[None, :],
                      np.float32(NEGT), np.float32(0.0)).astype(NPBF16)

    sqrtD = math.sqrt(D)
    # one-hot selector: oh[h*T + r, t*128 + c] = 1 if r == t else 0
    ohblk = np.repeat(np.eye(T, dtype=np.float32), TILE, axis=1)
    oh = np.vstack([ohblk, ohblk]).astype(NPBF16)
    in_maps = []
    for c in range(NCORES):
        heads = [c * HPC + i for i in range(HPC)]
        wqk_c = np.concatenate(
            [wq[:, hh * D:(hh + 1) * D] for hh in heads]
            + [wk[:, hh * D:(hh + 1) * D] for hh in heads], axis=1)
        wv_c = np.concatenate([wv[:, hh * D:(hh + 1) * D] for hh in heads],
                              axis=1)
        wo_c = np.concatenate([wo[hh * D:(hh + 1) * D, :] for hh in heads],
                              axis=0)
        # bias rows: partition h*T+t holds B(t, v) repeated over the 128
        # columns of segment v; B = sqrt(D)*ln(m) (m>0) or -4e9 (m=0).
        mwx = np.zeros((2 * T, T * TILE), np.float32)
        for i, hh in enumerate(heads):
            for t in range(T):
                sel = list(anchor_indices[0, hh, t]) + [t]
                rowvals = np.full(T, np.float32(NEGB), np.float32)
                for v in range(t + 1):
                    m = sel.count(v)
                    if m:
                        rowvals[v] = sqrtD * math.log(m)
                mwx[i * T + t, :] = np.repeat(rowvals, TILE)
        in_maps.append({
            "xT": xT, "wqk": np.ascontiguousarray(wqk_c).astype(NPBF16),
            "wv": np.ascontiguousarray(wv_c).astype(NPBF16),
            "wo": np.ascontiguousarray(wo_c).astype(NPBF16),
            "cosT": cosT, "sinT": sinT, "rotT": rotT,
            "identD": ident, "trinegD": trineg,
            "mwxD": mwx.astype(NPBF16), "ohD": oh,
        })
    return in_maps


def kernel(x, wq, wk, wv, wo, rope_angles, anchor_indices, **run_kwargs):
    _patch_tile_drain()
    nc = build_bass()
    in_maps = _host_prep(x, wq, wk, wv, wo, rope_angles, anchor_indices)
    res = run_bass_kernel_spmd(nc, in_maps, core_ids=list(range(NCORES)),
                               **run_kwargs)
    acc = np.zeros((E, S), np.float64)
    for c in range(NCORES):
        acc += res.results[c]["outT"].astype(np.float64)
    out = np.ascontiguousarray(acc.T.reshape(B, S, E)).astype(np.float32)
    kernel.last_results = res
    return out
